# revision 1
# baseline (speedup 1.0000x reference)
"""Fused CIN-layer kernel for Trainium2 (8 NeuronCores, batch data-parallel).

True reference semantics (derived from the row-major .view + strided conv):
  out[b, n, c*32+t] = sum_{i<32, y<32} W[n,i,y] * x0[b,t,2i+c] * xk[b,y,2i+c] + bias[n]
where c in {0,1} is the f-parity and i indexes f-pairs.

Per core (128 batches, bc = b_local*2 + c in [0,256), groups J of 4 bc's):
  stage1 (PE):  per i: G_i[n, bc] = sum_y W[n,i,y] * xk[b,y,2i+c]
                i = 4q + r; the 4 r-matmuls of a quad run CONCURRENTLY via
                tile_position=(32r, 0) row tiling (lhsT/rhs live at
                partitions 32r..32r+31).  Concurrent row tiles must drain
                to DISTINCT PSUM banks; and readers of one tile are chained
                by the Tile scheduler, so the quad PSUM is two tiles
                (gqa: r 0-1, gqb: r 2-3) evacuated concurrently by ACT/DVE
                -> Gsb[n, bc*32+i] fp16.
  transpose (PE): per J: Gt_J[(j,i), n] = Gsb^T via PE transpose, fp16
                PSUM, evac alternating DVE/ACT.
  stage2 (PE):  out_J[(j,t), n] = sum_{(j,i)} X0bd_J[(j,i),(j,t)] * Gt_J[(j,i),n]
                X0bd = host-built block-diagonal x0 tiles (fp16).
                PSUM fp32 -> fp16 osb (alternating ACT/DVE), per-J8 128KB
                output DMA; host adds bias + final reshape.
  DMA:          wst+iden+xks merged into one input tensor (2 chunked
                transfers on the Sync HWDGE queue); x0a rides the Scalar
                HWDGE queue in parallel.  Output fp16 (halves the tail).
  PSUM budget:  stage-1 pool (2 tags x 2 bufs x 2 banks = 8 banks) closes
                before the gt/po pools open (stack allocator reuse).
"""

import numpy as np

BS, T, Y, F, NF = 1024, 32, 32, 64, 64
NCORES = 8
BPC = BS // NCORES      # 128 batches per core
NBC = BPC * 2           # 256 (b,c) pairs per core
NG = NBC // 4           # 64 groups of 4
NI = 32                 # f-pair index
NQ = NI // 4            # 8 stage-1 quads

W_OFF = 0               # wst at win[:, 0:512]
I_OFF = NQ * NF         # iden at win[:, 512:576]
K_OFF = I_OFF + NF      # xks at win[:, 576:2624]
WIN_W = K_OFF + NQ * NBC

_cached = {}


def _build_bass():
    import concourse.bass as bass
    import concourse.mybir as mybir
    from concourse import bacc
    from concourse.tile import TileContext

    F16 = mybir.dt.float16
    F32 = mybir.dt.float32

    nc = bacc.Bacc()
    # merged input: [wst (r,y)x(q,n) | iden | xks (r,y)x(q,bc)]
    win = nc.dram_tensor("win", [128, WIN_W], F16, kind="ExternalInput")
    # block-diagonal x0: partition (j, i); col (J, j2, t)
    x0a = nc.dram_tensor("x0a", [128, NG * 128], F16, kind="ExternalInput")
    # out fp16: partition (j, t); col (J, n)
    outd = nc.dram_tensor("outd", [128, NG * NF], F16, kind="ExternalOutput")

    with TileContext(nc) as tc:
        with (
            tc.tile_pool(name="const", bufs=1) as cpool,
            tc.tile_pool(name="sb", bufs=1) as spool,
        ):
            win_sb = cpool.tile([128, WIN_W], F16)
            # Two HWDGE queues run ~2x one queue's rate, but the big x0a
            # stream must not compete with the stage1-critical chunks:
            # A (quads 0-3) and B (quads 4-7) ride the Scalar queue alone;
            # x0a rides the Sync queue but is gated behind chunk B by an
            # artificial WAW dep (the tiny DVE copy writes into x0a's c1
            # dst range and reads the tail of chunk B).
            ca = K_OFF + 4 * NBC
            nc.sync.dma_start(out=win_sb[:, :ca], in_=win[:, :ca])
            nc.sync.dma_start(out=win_sb[:, ca:], in_=win[:, ca:])
            x0a_sb = cpool.tile([128, NG * 128], F16)
            for c0, c1 in ((0, 3072), (3072, 6144), (6144, 8192)):
                nc.gpsimd.dma_start(out=x0a_sb[:, c0:c1], in_=x0a[:, c0:c1])

            wst_sb = win_sb[:, W_OFF:W_OFF + NQ * NF]
            id_sb = win_sb[0:NF, I_OFF:I_OFF + NF]

            gsb = spool.tile([NF, NBC * NI], F16)    # G[n, bc*32+i]
            gt_sb = spool.tile([128, NG * NF], F16)  # Gt[(j,i), J*64+n]
            osb = spool.tile([128, NG * NF], F16)    # out[(j,t), J*64+n]

            # stage 1: 8 quads; quad q covers i = 4q+r with 4 row-tiled
            # concurrent matmuls, one PSUM bank each -> Gsb
            with tc.tile_pool(name="gq", bufs=2, space="PSUM") as gqpool:
                for q in range(NQ):
                    gqa = gqpool.tile([NF, 2 * 512], mybir.dt.float32, tag="gqa")
                    gqb = gqpool.tile([NF, 2 * 512], mybir.dt.float32, tag="gqb")
                    halves = [gqa, gqa, gqb, gqb]
                    for r in range(4):
                        nc.tensor.matmul(
                            halves[r][:, (r % 2) * 512:(r % 2) * 512 + NBC],
                            wst_sb[32 * r:32 * r + 32, q * NF:(q + 1) * NF],
                            win_sb[32 * r:32 * r + 32,
                                   K_OFF + q * NBC:K_OFF + (q + 1) * NBC],
                            start=True, stop=True,
                            tile_position=(32 * r, 0),
                        )
                    # evac to Gsb[n, bc*32 + 4q + r]; iteration (bc, r)
                    out_ap = gsb[:, :].rearrange(
                        "p (bc i) -> p bc i", bc=NBC, i=NI)[:, :, 4 * q:4 * q + 4]
                    in_a = gqa[:, :].rearrange(
                        "p (r w) -> p w r", r=2, w=512)[:, :NBC, :]
                    in_b = gqb[:, :].rearrange(
                        "p (r w) -> p w r", r=2, w=512)[:, :NBC, :]
                    nc.scalar.copy(out_ap[:, :, 0:2], in_a)
                    nc.vector.tensor_copy(out_ap[:, :, 2:4], in_b)

            with (
                tc.tile_pool(name="gt", bufs=3, space="PSUM") as gtpool,
                tc.tile_pool(name="po", bufs=3, space="PSUM") as popool,
            ):
                # per J8: 8 transposes -> gt evac -> 8 stage-2 matmuls ->
                # po evac -> 128KB output DMA
                for J8 in range(NG // 8):
                    gt8 = gtpool.tile([128, 8 * NF], F16, tag="gt8")
                    for s in range(8):
                        J = J8 * 8 + s
                        nc.tensor.transpose(
                            gt8[:, s * NF:(s + 1) * NF],
                            gsb[:, J * 128:(J + 1) * 128],
                            id_sb[:, :],
                        )
                    o0 = J8 * 8 * NF
                    # gt evac gates the stage-2 matmuls; DVE runs fp16
                    # PSUM reads at 2x (425ns vs ACT 687ns), so DVE takes
                    # all of them and ACT absorbs most po evacs below
                    nc.vector.tensor_copy(gt_sb[:, o0:o0 + 8 * NF], gt8[:, :])

                    po = popool.tile([128, 8 * NF], mybir.dt.float32, tag="po")
                    for s in range(8):
                        J = J8 * 8 + s
                        nc.tensor.matmul(
                            po[:, s * NF:(s + 1) * NF],
                            x0a_sb[:, J * 128:(J + 1) * 128],
                            gt_sb[:, J * NF:(J + 1) * NF],
                            start=True, stop=True,
                        )
                    if J8 % 4 == 3:
                        nc.vector.tensor_copy(osb[:, o0:o0 + 8 * NF], po[:, :])
                    else:
                        nc.scalar.copy(osb[:, o0:o0 + 8 * NF], po[:, :])
                    # chunked output: [0,1] [2,3] [4,5] [6] [7] — the small
                    # final chunks shorten the post-compute DMA drain
                    if J8 in (1, 3, 5):
                        d0 = (J8 - 1) * 8 * NF
                        nc.sync.dma_start(out=outd[:, d0:o0 + 8 * NF],
                                          in_=osb[:, d0:o0 + 8 * NF])
                    elif J8 >= 6:
                        nc.sync.dma_start(out=outd[:, o0:o0 + 8 * NF],
                                          in_=osb[:, o0:o0 + 8 * NF])
    nc.compile()
    return nc


def _host_prep(x_0, x_k, weight):
    f16 = np.float16
    x_0 = np.asarray(x_0, dtype=np.float32)
    x_k = np.asarray(x_k, dtype=np.float32)
    W = np.asarray(weight, dtype=np.float32).reshape(NF, NI, Y)

    # wst[32r+y, q*64+n] = W[n, 4q+r, y]
    Wr = W.reshape(NF, NQ, 4, Y)                      # n, q, r, y
    wstn = Wr.transpose(2, 3, 1, 0).reshape(128, NQ * NF)
    iden = np.zeros((128, NF), np.float32)
    iden[:NF] = np.eye(NF, dtype=np.float32)

    win_l, x0a_l = [], []
    jj = np.arange(4)
    for core in range(NCORES):
        xkc = x_k[core * BPC:(core + 1) * BPC]        # [128, y, f]
        x0c = x_0[core * BPC:(core + 1) * BPC]        # [128, t, f]
        # xks[32r+y, q*256 + b_l*2 + c] = xk[b_l, y, 2(4q+r)+c]
        xkr = xkc.reshape(BPC, Y, NQ, 4, 2)           # b_l, y, q, r, c
        xksn = xkr.transpose(3, 1, 2, 0, 4).reshape(128, NQ * NBC)
        win = np.concatenate([wstn, iden, xksn], axis=1)
        win_l.append(np.ascontiguousarray(win).astype(f16))
        # x0 per bc: [bc, i, t]
        x0r = x0c.reshape(BPC, T, NI, 2)              # b_l, t, i, c
        x0bc = x0r.transpose(0, 3, 2, 1).reshape(NBC, NI, T)
        # block-diagonal tiles: X0bd[J, j, i, j2, t] = delta(j,j2)*x0bc[4J+j, i, t]
        x0bd = np.zeros((NG, 4, NI, 4, T), dtype=np.float32)
        x0bd[:, jj, :, jj, :] = x0bc.reshape(NG, 4, NI, T).transpose(1, 0, 2, 3)
        # rows (j, i), cols (J, j2, t)
        x0a = x0bd.transpose(1, 2, 0, 3, 4).reshape(128, NG * 128)
        x0a_l.append(np.ascontiguousarray(x0a).astype(f16))

    return win_l, x0a_l


def kernel(x_0, x_k, weight, bias):
    from concourse import bass_utils

    if "nc" not in _cached:
        _cached["nc"] = _build_bass()
    nc = _cached["nc"]

    win_l, x0a_l = _host_prep(x_0, x_k, weight)
    in_maps = [{"win": win_l[c], "x0a": x0a_l[c]} for c in range(NCORES)]
    res = bass_utils.run_bass_kernel_spmd(nc, in_maps, core_ids=list(range(NCORES)))

    bias = np.asarray(bias, dtype=np.float32)
    outs = []
    for c in range(NCORES):
        od = res.results[c]["outd"].astype(np.float32)  # [128=(j,t), NG*64=(J,n)]
        o = od.reshape(4, T, NG, NF)                # [j, t, J, n]
        o = o.transpose(2, 0, 3, 1)                 # [J, j, n, t]
        o = o.reshape(BPC, 2, NF, T)                # [b_l, c, n, t]
        o = o.transpose(0, 2, 1, 3).reshape(BPC, NF, 2 * T)  # [b_l, n, c*32+t]
        outs.append(o)
    out = np.concatenate(outs, axis=0)
    out = out + bias[None, :, None]
    return np.ascontiguousarray(out.astype(np.float32))



# revision 2
# speedup vs baseline: 1.1116x; 1.1116x over previous
"""Fused CIN-layer kernel for Trainium2 (8 NeuronCores, batch data-parallel).

True reference semantics (derived from the row-major .view + strided conv):
  out[b, n, c*32+t] = sum_{i<32, y<32} W[n,i,y] * x0[b,t,2i+c] * xk[b,y,2i+c] + bias[n]
where c in {0,1} is the f-parity and i indexes f-pairs.

v2 layout: the f-parity c is packed into PSUM/SBUF PARTITION halves
(64c + n) instead of being interleaved along columns.  Per core
(128 batches b_l; groups g = (J', c) of 4 b_l's):

  stage1 (PE):  per quad q (i = 4q+r): 8 matmuls (r x c), each
                G[(c,n), b_l] = sum_y W[n,i,y] * xk[b_l,y,2i+c] via
                tile_position=(32r, 64c): row strip r contracts y,
                column strip c places the output at partitions 64c..64c+63.
                r0/r1 -> gqa (2 banks), r2/r3 -> gqb; concurrent row
                strips land in distinct banks; the c-pair of one strip
                serializes (shared stream path) so same-bank is safe.
  evac:         2 copies per quad ([128, 2x128], full 128 lanes) ->
                gsb[(c,n), b_l*32+i] fp16 (ACT takes gqa, DVE gqb).
  transpose(PE):per J' (4 b_l's): [128,128] PE transpose of
                gsb[:, J'*128:+128] -> Gt[(j,i), (c,n)], fp16 PSUM,
                4 per gt8 tile, DVE evac -> gt_sb[(j,i), g*64+n].
  stage2 (PE):  per g: out[(j,t), n] = X0bd_g^T @ Gt_g with
                X0bd = host-built block-diagonal x0 (fp16, 128x128).
                PSUM fp32 -> fp16 osb (ACT/DVE alternating), chunked
                output DMA; host adds bias + final reshape.
  DMA:          win (wst|iden|xks) and x0a all ride the Sync HWDGE
                queue: winA, winB, then 3 x0a chunks - FIFO gives the
                stage1-critical chunks natural priority.  Output fp16.
  PSUM budget:  stage-1 pool (2 tags x 2 bufs x 2 banks = 8 banks)
                closes before the gt/po pools open.
"""

import numpy as np

BS, T, Y, F, NF = 1024, 32, 32, 64, 64
NCORES = 8
BPC = BS // NCORES      # 128 batches per core
NI = 32                 # f-pair index
NQ = NI // 4            # 8 stage-1 quads
NG = 64                 # stage-2 groups g = (J', c), 4 b_l's each
NJP = 32                # J' index (b_l // 4)

W_OFF = 0               # wst at win[:, 0:512]
I_OFF = NQ * NF         # iden at win[:, 512:640]
K_OFF = I_OFF + 128     # xks at win[:, 640:2688]
WIN_W = K_OFF + NQ * 256
CHUNK_A = K_OFF + 4 * 256   # wst + iden + xks quads 0-3

_cached = {}


def _build_bass():
    import concourse.bass as bass
    import concourse.mybir as mybir
    from concourse import bacc
    from concourse.tile import TileContext

    F16 = mybir.dt.float16
    F32 = mybir.dt.float32

    nc = bacc.Bacc()
    # merged input: [wst (r,y)x(q,n) | iden128 | xks (r,y)x(q,c,b_l)]
    win = nc.dram_tensor("win", [128, WIN_W], F16, kind="ExternalInput")
    # block-diagonal x0: partition (j, i); col (J', c, j2, t)
    x0a = nc.dram_tensor("x0a", [128, NG * 128], F16, kind="ExternalInput")
    # out fp16: partition (j, t); col (J', c, n)
    outd = nc.dram_tensor("outd", [128, NG * NF], F16, kind="ExternalOutput")

    with TileContext(nc) as tc:
        with (
            tc.tile_pool(name="const", bufs=1) as cpool,
            tc.tile_pool(name="sb", bufs=1) as spool,
        ):
            win_sb = cpool.tile([128, WIN_W], F16)
            x0a_sb = cpool.tile([128, NG * 128], F16)
            # All input on the Sync HWDGE ring, FIFO order = priority order:
            # winA (stage-1 quads 0-3) -> winB -> x0a chunks (stage-2).
            nc.sync.dma_start(out=win_sb[:, :CHUNK_A], in_=win[:, :CHUNK_A])
            nc.sync.dma_start(out=win_sb[:, CHUNK_A:], in_=win[:, CHUNK_A:])
            for c0, c1 in ((0, 3072), (3072, 6144), (6144, 8192)):
                nc.sync.dma_start(out=x0a_sb[:, c0:c1], in_=x0a[:, c0:c1])

            wst_sb = win_sb[:, W_OFF:W_OFF + NQ * NF]
            id_sb = win_sb[0:128, I_OFF:I_OFF + 128]

            gsb = spool.tile([128, BPC * NI], F16)   # G[(c,n), b_l*32+i]
            gt_sb = spool.tile([128, NG * NF], F16)  # Gt[(j,i), g*64+n]
            osb = spool.tile([128, NG * NF], F16)    # out[(j,t), g*64+n]

            # stage 1: 8 quads; quad q covers i = 4q+r; per (r, c) one
            # 32-row matmul at tile_position (32r, 64c).  gqa holds r 0-1
            # (one bank each at col 0/512), gqb r 2-3.
            with tc.tile_pool(name="gq", bufs=2, space="PSUM") as gqpool:
                for q in range(NQ):
                    gqa = gqpool.tile([128, 1024], F32, tag="gqa")
                    gqb = gqpool.tile([128, 1024], F32, tag="gqb")
                    for r in range(4):
                        tl = gqa if r < 2 else gqb
                        col = (r % 2) * 512
                        for c in range(2):
                            nc.tensor.matmul(
                                tl[c * 64:(c + 1) * 64, col:col + BPC],
                                wst_sb[32 * r:32 * r + 32, q * NF:(q + 1) * NF],
                                win_sb[32 * r:32 * r + 32,
                                       K_OFF + q * 256 + c * BPC:
                                       K_OFF + q * 256 + (c + 1) * BPC],
                                start=True, stop=True,
                                tile_position=(32 * r, 64 * c),
                            )
                    # evac to gsb[(c,n), b_l*32 + 4q + rb]
                    dst = gsb[:, :].rearrange(
                        "p (b i) -> p b i", b=BPC, i=NI)
                    in_a = gqa[:, :].rearrange(
                        "p (rb w) -> p w rb", rb=2, w=512)[:, :BPC, :]
                    in_b = gqb[:, :].rearrange(
                        "p (rb w) -> p w rb", rb=2, w=512)[:, :BPC, :]
                    nc.scalar.copy(dst[:, :, 4 * q:4 * q + 2], in_a)
                    nc.vector.tensor_copy(dst[:, :, 4 * q + 2:4 * q + 4], in_b)

            with (
                tc.tile_pool(name="gt", bufs=3, space="PSUM") as gtpool,
                tc.tile_pool(name="po", bufs=3, space="PSUM") as popool,
            ):
                # per J8 (8 groups g = 4 J'): 4 transposes -> gt evac ->
                # 8 stage-2 matmuls -> po evac -> chunked output DMA
                for J8 in range(8):
                    gt8 = gtpool.tile([128, 512], F16, tag="gt8")
                    for s in range(4):
                        Jp = J8 * 4 + s
                        nc.tensor.transpose(
                            gt8[:, s * 128:(s + 1) * 128],
                            gsb[:, Jp * 128:(Jp + 1) * 128],
                            id_sb[:, :],
                        )
                    o0 = J8 * 8 * NF
                    nc.vector.tensor_copy(gt_sb[:, o0:o0 + 512], gt8[:, :])

                    po = popool.tile([128, 512], F32, tag="po")
                    for s2 in range(8):
                        g = J8 * 8 + s2
                        nc.tensor.matmul(
                            po[:, s2 * NF:(s2 + 1) * NF],
                            x0a_sb[:, g * 128:(g + 1) * 128],
                            gt_sb[:, g * NF:(g + 1) * NF],
                            start=True, stop=True,
                        )
                    if J8 % 4 == 3:
                        nc.vector.tensor_copy(osb[:, o0:o0 + 512], po[:, :])
                    else:
                        nc.scalar.copy(osb[:, o0:o0 + 512], po[:, :])
                    # chunked output: [0,1] [2,3] [4,5] [6] [7]
                    if J8 in (1, 3, 5):
                        d0 = (J8 - 1) * 8 * NF
                        nc.sync.dma_start(out=outd[:, d0:o0 + 512],
                                          in_=osb[:, d0:o0 + 512])
                    elif J8 >= 6:
                        nc.sync.dma_start(out=outd[:, o0:o0 + 512],
                                          in_=osb[:, o0:o0 + 512])
    nc.compile()
    return nc


def _host_prep(x_0, x_k, weight):
    f16 = np.float16
    x_0 = np.asarray(x_0, dtype=np.float32)
    x_k = np.asarray(x_k, dtype=np.float32)
    W = np.asarray(weight, dtype=np.float32).reshape(NF, NI, Y)

    # wst[32r+y, q*64+n] = W[n, 4q+r, y]
    Wr = W.reshape(NF, NQ, 4, Y)                      # n, q, r, y
    wstn = Wr.transpose(2, 3, 1, 0).reshape(128, NQ * NF)
    iden = np.eye(128, dtype=np.float32)

    win_l, x0a_l = [], []
    jj = np.arange(4)
    for core in range(NCORES):
        xkc = x_k[core * BPC:(core + 1) * BPC]        # [128, y, f]
        x0c = x_0[core * BPC:(core + 1) * BPC]        # [128, t, f]
        # xks[32r+y, q*256 + c*128 + b_l] = xk[b_l, y, 2(4q+r)+c]
        xkr = xkc.reshape(BPC, Y, NQ, 4, 2)           # b_l, y, q, r, c
        xksn = xkr.transpose(3, 1, 2, 4, 0).reshape(128, NQ * 256)
        win = np.concatenate([wstn, iden, xksn], axis=1)
        win_l.append(np.ascontiguousarray(win).astype(f16))
        # x0 per (c, J', j): [c, J', j, i, t]
        x0r = x0c.reshape(BPC, T, NI, 2)              # b_l, t, i, c
        A = x0r.transpose(3, 0, 2, 1).reshape(2, NJP, 4, NI, T)
        # block-diagonal: x0bd[c, J', j, i, j2, t] = delta(j,j2)*A[c,J',j,i,t]
        x0bd = np.zeros((2, NJP, 4, NI, 4, T), dtype=np.float32)
        x0bd[:, :, jj, :, jj, :] = A.transpose(2, 0, 1, 3, 4)
        # rows (j, i); cols (J', c, j2, t)
        x0an = x0bd.transpose(2, 3, 1, 0, 4, 5).reshape(128, NG * 128)
        x0a_l.append(np.ascontiguousarray(x0an).astype(f16))

    return win_l, x0a_l


def kernel(x_0, x_k, weight, bias):
    from concourse import bass_utils

    if "nc" not in _cached:
        _cached["nc"] = _build_bass()
    nc = _cached["nc"]

    win_l, x0a_l = _host_prep(x_0, x_k, weight)
    in_maps = [{"win": win_l[c], "x0a": x0a_l[c]} for c in range(NCORES)]
    res = bass_utils.run_bass_kernel_spmd(nc, in_maps, core_ids=list(range(NCORES)))

    bias = np.asarray(bias, dtype=np.float32)
    outs = []
    for c in range(NCORES):
        od = res.results[c]["outd"].astype(np.float32)  # [(j,t), (J',c,n)]
        o = od.reshape(4, T, NJP, 2, NF)            # [j, t, J', c, n]
        o = o.transpose(2, 0, 4, 3, 1)              # [J', j, n, c, t]
        o = o.reshape(BPC, NF, 2 * T)               # [b_l, n, c*32+t]
        outs.append(o)
    out = np.concatenate(outs, axis=0)
    out = out + bias[None, :, None]
    return np.ascontiguousarray(out.astype(np.float32))
